# revision 1
# baseline (speedup 1.0000x reference)
"""GQA attention kernel for Trainium2, sharded over 8 NeuronCores.

Problem: X (1, 4096, 1024), H=16 q-heads, KVH=4 kv-heads, head_dim=64.
Sharding: 2 q-heads + their shared kv-head per core (tensor parallel over H).
Each core computes q/k/v projections for its heads, fused flash-style
attention (scores never leave PSUM/SBUF), and its 128-row slice of the
output projection -> partial (4096, 1024) f32, summed on host.

Layouts on device (per core):
  xt   : X^T            (1024 D, 4096 S)  bf16   (host pre-transposed)
  qt   : Q^T            (128 = 2 heads x 64 d, 4096 q) bf16
  kvt  : [K^T; V^T]     (128 = 64 k-d + 64 v-d, 4096 s) bf16
  v    : V natural+ones (128 s-tile, 65) x 32 tiles bf16 (col 64 == 1.0)
  St   : scores^T       (128 k, QC q) f32 PSUM  = Kt_tile.T @ Qt
  Pt   : exp(St/8)      (128 k, QC q) bf16 SBUF (ScalarE, scale folded)
  Ot   : V_aug.T @ Pt   (65, QC) f32 PSUM; row 64 = softmax denominators
  y    : partial output (4096, 1024) f32 = (Ot/denoms).T @ o_w[rows]
"""

import sys

import numpy as np

try:
    import concourse.bass as bass
except ImportError:  # grading env may not have concourse on sys.path
    for p in ("/opt/trn_rl_repo", "/root/.axon_site/_ro/trn_rl_repo"):
        if p not in sys.path:
            sys.path.append(p)
    import concourse.bass as bass

import bass_rust
import ml_dtypes
from concourse import mybir
from concourse.bass_utils import run_bass_kernel_spmd
from concourse.masks import make_identity
from concourse.tile import TileContext

BF16 = ml_dtypes.bfloat16

B, S, D = 1, 4096, 1024
H, KVH, HD = 16, 4, 64
NCORES = 8
HPC = H // NCORES          # 2 q heads per core
DQ = HPC * HD              # 128 projected q dims per core
DKV = 2 * HD               # 128 = k head + v head dims
QC = 1024                  # attention q-chunk (PSUM tile free size)
KT = 128                   # k tile (seq positions per score tile)
NKT = S // KT              # 32
NQC = S // QC              # 4
NDC = D // 128             # 8 contraction chunks for projections
MM_N = 512                 # max matmul free dim (one PSUM bank, f32)

_COMPILED = None


def build_bass(reps=1):
    nc = bass.Bass()
    fp32 = mybir.dt.float32
    bf16 = mybir.dt.bfloat16

    xt = nc.declare_dram_parameter("xt", [D, S], bf16, isOutput=False)
    qw = nc.declare_dram_parameter("qw", [D, DQ], bf16, isOutput=False)
    kvw = nc.declare_dram_parameter("kvw", [D, DKV], bf16, isOutput=False)
    ow = nc.declare_dram_parameter("ow", [DQ, D], bf16, isOutput=False)
    qb = nc.declare_dram_parameter("qb", [1, DQ], bf16, isOutput=False)
    kvb = nc.declare_dram_parameter("kvb", [1, DKV], bf16, isOutput=False)
    y = nc.declare_dram_parameter("y", [S, D], fp32, isOutput=True)

    with TileContext(nc) as tc:
        with (
            tc.tile_pool(name="singles", bufs=1) as singles,
            tc.tile_pool(name="pt_pool", bufs=3) as pt_pool,
            tc.tile_pool(name="bc_pool", bufs=2) as bc_pool,
            tc.tile_pool(name="ysb", bufs=3) as ysb_pool,
            tc.tile_pool(name="dramp", bufs=2, space="DRAM") as dram_pool,
            tc.tile_pool(name="ps_st", bufs=2, space="PSUM") as ps_st,
            tc.tile_pool(name="ps_ot", bufs=2, space="PSUM") as ps_ot,
        ):
            # ---- constants / weights ----
            ident = singles.tile([128, 128], bf16)
            make_identity(nc, ident)

            qw_sb = singles.tile([128, NDC, DQ], bf16)
            nc.sync.dma_start(
                out=qw_sb, in_=qw[:, :].rearrange("(c p) m -> p c m", p=128)
            )
            kvw_sb = singles.tile([128, NDC, DKV], bf16)
            nc.sync.dma_start(
                out=kvw_sb, in_=kvw[:, :].rearrange("(c p) m -> p c m", p=128)
            )
            ow_sb = singles.tile([DQ, D], bf16)
            nc.sync.dma_start(out=ow_sb, in_=ow[:, :])
            qb_sb = singles.tile([1, DQ], bf16)
            nc.sync.dma_start(out=qb_sb, in_=qb[:, :])
            kvb_sb = singles.tile([1, DKV], bf16)
            nc.sync.dma_start(out=kvb_sb, in_=kvb[:, :])
            ones_row = singles.tile([1, MM_N], bf16)
            nc.vector.memset(ones_row, 1.0)

            xt_sb = singles.tile([128, NDC, S], bf16)
            for j in range(S // MM_N):
                nc.sync.dma_start(
                    out=xt_sb[:, :, bass.ts(j, MM_N)],
                    in_=xt[:, :].rearrange("(c p) s -> p c s", p=128)[
                        :, :, bass.ts(j, MM_N)],
                )

            for _rep in range(reps):
                # ---- projections: Qt and KVt (transposed: head-dims on partitions) ----
                qt_sb = singles.tile([DQ, S], bf16)
                kvt_sb = singles.tile([DKV, S], bf16)
                for j in range(S // MM_N):
                    sl = bass.ts(j, MM_N)
                    psq = ps_st.tile([128, MM_N], fp32, tag="ps_st")
                    for c in range(NDC):
                        nc.tensor.matmul(
                            psq, qw_sb[:, c, :], xt_sb[:, c, sl],
                            start=(c == 0), stop=False,
                        )
                    nc.tensor.matmul(psq, qb_sb, ones_row, start=False, stop=True)
                    nc.vector.tensor_copy(qt_sb[:, sl], psq)
                    pskv = ps_st.tile([128, MM_N], fp32, tag="ps_st")
                    for c in range(NDC):
                        nc.tensor.matmul(
                            pskv, kvw_sb[:, c, :], xt_sb[:, c, sl],
                            start=(c == 0), stop=False,
                        )
                    nc.tensor.matmul(pskv, kvb_sb, ones_row, start=False, stop=True)
                    nc.vector.tensor_copy(kvt_sb[:, sl], pskv)

                # Kt duplicated into both partition halves (score matmul operands
                # must share a base partition with each head's q rows)
                kt2_sb = singles.tile([DKV, S], bf16)
                nc.sync.dma_start(out=kt2_sb[0:HD, :], in_=kvt_sb[0:HD, :])
                nc.sync.dma_start(out=kt2_sb[HD:DKV, :], in_=kvt_sb[0:HD, :])

                # ---- V into natural layout (s on partitions) + ones column ----
                v_sb = singles.tile([128, NKT, HD + 1], bf16)
                nc.vector.memset(v_sb, 1.0)
                for t in range(NKT):
                    pvt = ps_st.tile([128, HD], bf16, tag="ps_st")
                    nc.tensor.transpose(
                        pvt, kvt_sb[HD:DKV, bass.ts(t, KT)], ident[HD:DKV, HD:DKV]
                    )
                    nc.vector.tensor_copy(v_sb[:, t, 0:HD], pvt)

                # ---- attention (flash-style, scores transposed) ----
                ot_full = singles.tile([DQ, S], bf16)  # normalized attn out, d' on parts
                for jc in range(NQC):
                    qsl = bass.ts(jc, QC)
                    ots = []
                    for h in range(HPC):
                        ot_h = ps_ot.tile([HD + 1, QC], fp32, tag="ps_ot")
                        ots.append(ot_h)
                    for t in range(NKT):
                        pts = []
                        for h in range(HPC):
                            st = ps_st.tile([128, QC], fp32, tag="ps_st")
                            for u in range(QC // MM_N):
                                nc.tensor.matmul(
                                    st[:, bass.ts(u, MM_N)],
                                    kt2_sb[h * HD:(h + 1) * HD, bass.ts(t, KT)],
                                    qt_sb[h * HD:(h + 1) * HD,
                                          jc * QC + u * MM_N:
                                          jc * QC + (u + 1) * MM_N],
                                    start=True, stop=True,
                                )
                            pt = pt_pool.tile([128, QC], bf16, tag="pt")
                            nc.scalar.activation(
                                pt, st, mybir.ActivationFunctionType.Exp,
                                scale=1.0 / 8.0,
                            )
                            pts.append(pt)
                        for h in range(HPC):
                            for u in range(QC // MM_N):
                                nc.tensor.matmul(
                                    ots[h][:, bass.ts(u, MM_N)],
                                    v_sb[:, t, :],
                                    pts[h][:, bass.ts(u, MM_N)],
                                    start=(t == 0), stop=(t == NKT - 1),
                                )
                    # normalize: rows 0..63 of ot divided by row 64 (denominators)
                    for h in range(HPC):
                        rs = bc_pool.tile([HD + 1, QC], fp32, tag="rs")
                        nc.vector.reciprocal(rs[HD:HD + 1, :], ots[h][HD:HD + 1, :])
                        bc = bc_pool.tile([HD, QC], fp32, tag="bc")
                        sc = dram_pool.tile([1, QC], fp32, tag="sc")
                        nc.gpsimd.dma_start(out=sc, in_=rs[HD:HD + 1, :])
                        sc_bcast = bass.AP(
                            tensor=sc.tensor, offset=sc.offset,
                            ap=[[0, HD], sc.ap[-1]],
                        )
                        nc.gpsimd.dma_start(out=bc, in_=sc_bcast)
                        nc.vector.tensor_mul(
                            ot_full[h * HD:(h + 1) * HD, qsl], ots[h][0:HD, :], bc
                        )
                    # ---- output projection for this q-chunk ----
                    for jq in range(jc * (QC // 128), (jc + 1) * (QC // 128)):
                        for u in range(D // MM_N):
                            yp = ps_st.tile([128, MM_N], fp32, tag="ps_st")
                            nc.tensor.matmul(
                                yp, ot_full[:, bass.ts(jq, 128)],
                                ow_sb[:, bass.ts(u, MM_N)],
                                start=True, stop=True,
                            )
                            ysb = ysb_pool.tile([128, MM_N], fp32, tag="ysb")
                            nc.vector.tensor_copy(ysb, yp)
                            nc.sync.dma_start(
                                out=y[:, :][bass.ts(jq, 128), bass.ts(u, MM_N)],
                                in_=ysb,
                            )
    _split_multi_waits(nc)
    return nc


def _split_multi_waits(nc):
    """This toolchain's walrus accepts at most one sync-wait per datapath
    instruction; move extra waits onto same-engine NoOps placed just before."""
    k = 0
    for f in nc.m.functions:
        for blk in f.blocks:
            out = []
            for inst in blk.instructions:
                si = getattr(inst, "sync_info", None)
                ow = list(si.on_wait) if (si and si.on_wait) else []
                if len(ow) > 1:
                    for w in ow[:-1]:
                        k += 1
                        nop = bass_rust.InstNoOp(
                            name=f"I-wsplit-{k}", ins=[], outs=[]
                        )
                        nop.engine = inst.engine
                        nop.sync_info = mybir.SyncInfo(
                            on_wait=[w], on_update=[]
                        )
                        out.append(nop)
                    inst.sync_info = mybir.SyncInfo(
                        on_wait=[ow[-1]], on_update=list(si.on_update or [])
                    )
                out.append(inst)
            blk.instructions = out


def _prep_inputs(X, q_w, q_b, k_w, k_b, v_w, v_b, o_w):
    Xt = np.ascontiguousarray(X.reshape(S, D).T).astype(BF16)
    in_maps = []
    for c in range(NCORES):
        kv = c // (NCORES // KVH)
        qs = slice(c * DQ, (c + 1) * DQ)
        ks = slice(kv * HD, (kv + 1) * HD)
        in_maps.append({
            "xt": Xt,
            "qw": np.ascontiguousarray(q_w[:, qs]).astype(BF16),
            "kvw": np.ascontiguousarray(
                np.concatenate([k_w[:, ks], v_w[:, ks]], axis=1)).astype(BF16),
            "ow": np.ascontiguousarray(o_w[qs, :]).astype(BF16),
            "qb": np.ascontiguousarray(q_b[qs]).reshape(1, DQ).astype(BF16),
            "kvb": np.ascontiguousarray(
                np.concatenate([k_b[ks], v_b[ks]])).reshape(1, DKV).astype(BF16),
        })
    return in_maps


def kernel(X, q_w, q_b, k_w, k_b, v_w, v_b, o_w, o_b, **run_kwargs):
    global _COMPILED
    if _COMPILED is None:
        _COMPILED = build_bass()
    in_maps = _prep_inputs(X, q_w, q_b, k_w, k_b, v_w, v_b, o_w)
    res = run_bass_kernel_spmd(
        _COMPILED, in_maps, list(range(NCORES)), **run_kwargs
    )
    parts = [r["y"] for r in res.results]
    out = parts[0].astype(np.float32)
    for p in parts[1:]:
        out = out + p
    out = out + o_b.astype(np.float32)[None, :]
    if run_kwargs:
        return out.reshape(B, S, D), res
    return out.reshape(B, S, D)



# revision 10
# speedup vs baseline: 1.4570x; 1.4570x over previous
"""GQA attention kernel for Trainium2, sharded over 8 NeuronCores.

Problem: X (1, 4096, 1024), H=16 q-heads, KVH=4 kv-heads, head_dim=64.
Sharding: 2 q-heads + their shared kv-head per core (tensor parallel over H).
Each core computes q/k/v projections for its heads, fused flash-style
attention, and its 128-row slice of the output projection -> partial
(4096, 1024) f32, summed on host.

v2: software-pipelined single instruction stream tuned to keep the PE
array continuously busy (it only reaches the 2.4 GHz p-state after ~3us
without stalls; the v1 kernel sat at 1.2 GHz through all of attention):
  - scores(t) are emitted before PV(t-1), so exp(t-1) on the Scalar
    engine overlaps score matmuls and PV never waits on it.
  - KV projection (chunk 0), Q projection (chunk c+1) and the output
    projection (chunk c-1) are spread through the attention loop as PE
    filler work instead of serial prologue/epilogue phases.
  - softmax normalization broadcasts the reciprocal denominators with a
    GpSimd partition_broadcast (v1 round-tripped through DRAM, stalling
    the PE ~20us per chunk).

Layouts on device (per core):
  xt   : X^T            (1024 D, 4096 S)  bf16   (host pre-transposed)
  qt   : Q^T            (128 = 2 heads x 64 d, 4096 q) bf16
  kvt  : [K^T; V^T]     (128 = 64 k-d + 64 v-d, 4096 s) bf16
  kt2  : K^T duplicated into both partition halves
  v    : V natural+ones (128 s-tile, 65) x 32 tiles bf16 (col 64 == 1.0)
  St   : scores^T       (128 k, 1024 q) f32 PSUM  = Kt_tile.T @ Qt
  Pt   : exp(St/8)      (128 k, 1024 q) bf16 SBUF (ScalarE, scale folded)
  Ot   : V_aug.T @ Pt   (65, 1024) f32 PSUM; row 64 = softmax denominators
  y    : partial output (4096, 1024) f32 = (Ot/denoms).T @ o_w[rows]
"""

import sys

import numpy as np

try:
    import concourse.bass as bass
except ImportError:  # grading env may not have concourse on sys.path
    for p in ("/opt/trn_rl_repo", "/root/.axon_site/_ro/trn_rl_repo"):
        if p not in sys.path:
            sys.path.append(p)
    import concourse.bass as bass

import bass_rust
import ml_dtypes
from concourse import mybir
from concourse.bass_utils import run_bass_kernel_spmd
from concourse.masks import make_identity
from concourse.tile import TileContext

BF16 = ml_dtypes.bfloat16

B, S, D = 1, 4096, 1024
H, KVH, HD = 16, 4, 64
NCORES = 8
HPC = H // NCORES          # 2 q heads per core
DQ = HPC * HD              # 128 projected q dims per core
DKV = 2 * HD               # 128 = k head + v head dims
QC = 1024                  # attention q-chunk (2 PSUM banks per score tile)
KT = 128                   # k tile (seq positions per score tile)
NKT = S // KT              # 32
NQC = S // QC              # 4
NDC = D // 128             # 8 contraction chunks for projections
MM_N = 512                 # max matmul free dim (one PSUM bank, f32)

_COMPILED = None


def build_bass():
    nc = bass.Bass()
    fp32 = mybir.dt.float32
    bf16 = mybir.dt.bfloat16

    xt = nc.declare_dram_parameter("xt", [D, S], bf16, isOutput=False)
    qw = nc.declare_dram_parameter("qw", [D, DQ], bf16, isOutput=False)
    kvw = nc.declare_dram_parameter("kvw", [D, DKV], bf16, isOutput=False)
    ow = nc.declare_dram_parameter("ow", [DQ, D], bf16, isOutput=False)
    qb = nc.declare_dram_parameter("qb", [DQ, 1], fp32, isOutput=False)
    kvb = nc.declare_dram_parameter("kvb", [DKV, 1], fp32, isOutput=False)
    y = nc.declare_dram_parameter("y", [S, D], fp32, isOutput=True)

    with TileContext(nc) as tc:
        with (
            tc.tile_pool(name="singles", bufs=1) as singles,
            tc.tile_pool(name="pt_pool", bufs=6) as pt_pool,
            tc.tile_pool(name="nrm", bufs=2) as nrm_pool,
            tc.tile_pool(name="ysb", bufs=3) as ysb_pool,
            tc.tile_pool(name="ps_st", bufs=2, space="PSUM") as ps_st,
            tc.tile_pool(name="ps_ot", bufs=2, space="PSUM") as ps_ot,
        ):
            # ---- constants / weights ----
            ident = singles.tile([128, 128], bf16)
            make_identity(nc, ident)

            qw_sb = singles.tile([128, NDC, DQ], bf16)
            nc.sync.dma_start(
                out=qw_sb, in_=qw[:, :].rearrange("(c p) m -> p c m", p=128)
            )
            kvw_sb = singles.tile([128, NDC, DKV], bf16)
            nc.sync.dma_start(
                out=kvw_sb, in_=kvw[:, :].rearrange("(c p) m -> p c m", p=128)
            )
            ow_sb = singles.tile([DQ, D], bf16)
            nc.sync.dma_start(out=ow_sb, in_=ow[:, :])
            qb_sb = singles.tile([DQ, 1], fp32)
            nc.sync.dma_start(out=qb_sb, in_=qb[:, :])
            kvb_sb = singles.tile([DKV, 1], fp32)
            nc.sync.dma_start(out=kvb_sb, in_=kvb[:, :])

            xt_sb = singles.tile([128, NDC, S], bf16)
            for j in range(S // MM_N):
                nc.sync.dma_start(
                    out=xt_sb[:, :, bass.ts(j, MM_N)],
                    in_=xt[:, :].rearrange("(c p) s -> p c s", p=128)[
                        :, :, bass.ts(j, MM_N)],
                )

            qt_sb = singles.tile([DQ, S], bf16)
            kvt_sb = singles.tile([DKV, S], bf16)
            kt2_sb = singles.tile([DKV, S], bf16)
            v_sb = singles.tile([128, NKT, HD + 1], bf16)
            nc.vector.memset(v_sb, 1.0)
            ot_full = singles.tile([DQ, S], bf16)
            ones_col = singles.tile([1, HD], bf16)
            nc.vector.memset(ones_col, 1.0)

            exp = mybir.ActivationFunctionType.Exp

            def proj_slice(dst, w_sb, b_sb, j):
                ps = ps_st.tile([128, MM_N], fp32, tag="st")
                for c2 in range(NDC):
                    nc.tensor.matmul(
                        ps, w_sb[:, c2, :], xt_sb[:, c2, bass.ts(j, MM_N)],
                        start=(c2 == 0), stop=(c2 == NDC - 1),
                    )
                nc.vector.tensor_scalar_add(
                    dst[:, bass.ts(j, MM_N)], ps, b_sb[:, 0:1]
                )

            def kv_slice(j):
                proj_slice(kvt_sb, kvw_sb, kvb_sb, j)
                nc.sync.dma_start(
                    out=kt2_sb[0:HD, bass.ts(j, MM_N)],
                    in_=kvt_sb[0:HD, bass.ts(j, MM_N)],
                )
                nc.sync.dma_start(
                    out=kt2_sb[HD:DKV, bass.ts(j, MM_N)],
                    in_=kvt_sb[0:HD, bass.ts(j, MM_N)],
                )
                for tt in range(4 * j, 4 * j + 4):
                    pvt = ps_st.tile([128, HD], bf16, tag="st")
                    nc.tensor.transpose(
                        pvt, kvt_sb[HD:DKV, bass.ts(tt, KT)],
                        ident[HD:DKV, HD:DKV],
                    )
                    nc.vector.tensor_copy(v_sb[:, tt, 0:HD], pvt)

            def emit_scores(c, t):
                res = []
                for h in range(HPC):
                    st = ps_st.tile([128, QC], fp32, tag="st")
                    for u in range(QC // MM_N):
                        nc.tensor.matmul(
                            st[:, bass.ts(u, MM_N)],
                            kt2_sb[h * HD:(h + 1) * HD, bass.ts(t, KT)],
                            qt_sb[h * HD:(h + 1) * HD,
                                  c * QC + u * MM_N:c * QC + (u + 1) * MM_N],
                            start=True, stop=True,
                        )
                    pt = pt_pool.tile([128, QC], bf16, tag="pt")
                    nc.scalar.activation(pt, st, exp, scale=1.0 / 8.0)
                    res.append(pt)
                return res

            ots = {}

            def emit_pv(pc, pt_, ppts):
                if pt_ == 0:
                    ot_a = ps_ot.tile([HD + 1, QC], fp32, tag="ot")
                    ot_b = ps_ot.tile([HD + 1, QC], fp32, tag="ot")
                    ots[pc] = (ot_a, ot_b)
                for o, pp in zip(ots[pc], ppts):
                    for u in range(QC // MM_N):
                        nc.tensor.matmul(
                            o[:, bass.ts(u, MM_N)], v_sb[:, pt_, :],
                            pp[:, bass.ts(u, MM_N)],
                            start=(pt_ == 0), stop=(pt_ == NKT - 1),
                        )

            def emit_norm(pc):
                # ot rows 0..63 / row 64 (denominators): reciprocal, round to
                # bf16, broadcast across the 64 hd partitions with a rank-1
                # PE matmul (ones column), copy to SBUF (DVE reads at most
                # one PSUM operand), multiply on DVE. Split into 512-wide
                # halves to shorten the critical chain at chunk boundaries.
                for h, o in enumerate(ots[pc]):
                    for u in range(QC // MM_N):
                        usl = bass.ts(u, MM_N)
                        rs = nrm_pool.tile([1, MM_N], fp32, tag="rs")
                        nc.vector.reciprocal(rs, o[HD:HD + 1, usl])
                        rsb = nrm_pool.tile([1, MM_N], bf16, tag="rsb")
                        nc.vector.tensor_copy(rsb, rs)
                        bc_ps = ps_st.tile([HD, MM_N], fp32, tag="st")
                        nc.tensor.matmul(
                            bc_ps, ones_col, rsb, start=True, stop=True
                        )
                        bc_sb = nrm_pool.tile([HD, MM_N], fp32, tag="bc")
                        nc.vector.tensor_copy(bc_sb, bc_ps)
                        nc.vector.tensor_mul(
                            ot_full[h * HD:(h + 1) * HD,
                                    pc * QC + u * MM_N:pc * QC + (u + 1) * MM_N],
                            o[0:HD, usl], bc_sb,
                        )
                del ots[pc]

            def outproj_piece(jq):
                for u2 in range(D // MM_N):
                    yp = ps_st.tile([128, MM_N], fp32, tag="st")
                    nc.tensor.matmul(
                        yp, ot_full[:, bass.ts(jq, 128)],
                        ow_sb[:, bass.ts(u2, MM_N)],
                        start=True, stop=True,
                    )
                    ysb = ysb_pool.tile([128, MM_N], fp32, tag="ysb")
                    nc.vector.tensor_copy(ysb, yp)
                    nc.sync.dma_start(
                        out=y[:, :][bass.ts(jq, 128), bass.ts(u2, MM_N)],
                        in_=ysb,
                    )

            # ---- prologue: first KV slice, V tiles 0-3, Q proj chunk 0 ----
            kv_slice(0)
            proj_slice(qt_sb, qw_sb, qb_sb, 0)
            proj_slice(qt_sb, qw_sb, qb_sb, 1)

            # ---- main software-pipelined loop ----
            OUTPROJ_STEPS = (2, 5, 8, 11, 14, 17, 20, 24)
            pending_outproj = []
            prev = None
            for s_ in range(NQC * NKT):
                c, t = divmod(s_, NKT)
                pts = emit_scores(c, t)
                if prev is not None:
                    (pc, pt_), ppts = prev
                    emit_pv(pc, pt_, ppts)
                    if pt_ == NKT - 1:
                        emit_norm(pc)
                        pending_outproj.extend(
                            range(pc * (QC // 128), (pc + 1) * (QC // 128))
                        )
                # ---- PE filler work (keeps the systolic array saturated) ----
                if c == 0 and t >= 1 and (t - 1) % 3 == 0:
                    j = (t - 1) // 3 + 1
                    if j < NDC:
                        kv_slice(j)
                if c + 1 < NQC:
                    if t == 6:
                        proj_slice(qt_sb, qw_sb, qb_sb, 2 * (c + 1))
                    elif t == 22:
                        proj_slice(qt_sb, qw_sb, qb_sb, 2 * (c + 1) + 1)
                if pending_outproj and t in OUTPROJ_STEPS:
                    outproj_piece(pending_outproj.pop(0))
                prev = ((c, t), pts)

            # ---- epilogue ----
            (pc, pt_), ppts = prev
            emit_pv(pc, pt_, ppts)
            emit_norm(pc)
            pending_outproj.extend(
                range(pc * (QC // 128), (pc + 1) * (QC // 128))
            )
            for jq in pending_outproj:
                outproj_piece(jq)
    _split_multi_waits(nc)
    return nc


def _split_multi_waits(nc):
    """This toolchain's walrus accepts at most one sync-wait per datapath
    instruction; move extra waits onto same-engine NoOps placed just before."""
    k = 0
    for f in nc.m.functions:
        for blk in f.blocks:
            out = []
            for inst in blk.instructions:
                si = getattr(inst, "sync_info", None)
                ow_ = list(si.on_wait) if (si and si.on_wait) else []
                if len(ow_) > 1:
                    for w in ow_[:-1]:
                        k += 1
                        nop = bass_rust.InstNoOp(
                            name=f"I-wsplit-{k}", ins=[], outs=[]
                        )
                        nop.engine = inst.engine
                        nop.sync_info = mybir.SyncInfo(
                            on_wait=[w], on_update=[]
                        )
                        out.append(nop)
                    inst.sync_info = mybir.SyncInfo(
                        on_wait=[ow_[-1]], on_update=list(si.on_update or [])
                    )
                out.append(inst)
            blk.instructions = out


def _prep_inputs(X, q_w, q_b, k_w, k_b, v_w, v_b, o_w):
    Xt = np.ascontiguousarray(X.reshape(S, D).T).astype(BF16)
    in_maps = []
    for c in range(NCORES):
        kv = c // (NCORES // KVH)
        qs = slice(c * DQ, (c + 1) * DQ)
        ks = slice(kv * HD, (kv + 1) * HD)
        in_maps.append({
            "xt": Xt,
            "qw": np.ascontiguousarray(q_w[:, qs]).astype(BF16),
            "kvw": np.ascontiguousarray(
                np.concatenate([k_w[:, ks], v_w[:, ks]], axis=1)).astype(BF16),
            "ow": np.ascontiguousarray(o_w[qs, :]).astype(BF16),
            "qb": np.ascontiguousarray(q_b[qs]).reshape(DQ, 1).astype(
                np.float32),
            "kvb": np.ascontiguousarray(
                np.concatenate([k_b[ks], v_b[ks]])).reshape(DKV, 1).astype(
                np.float32),
        })
    return in_maps


def kernel(X, q_w, q_b, k_w, k_b, v_w, v_b, o_w, o_b, **run_kwargs):
    global _COMPILED
    if _COMPILED is None:
        _COMPILED = build_bass()
    in_maps = _prep_inputs(X, q_w, q_b, k_w, k_b, v_w, v_b, o_w)
    res = run_bass_kernel_spmd(
        _COMPILED, in_maps, list(range(NCORES)), **run_kwargs
    )
    parts = [r["y"] for r in res.results]
    out = parts[0].astype(np.float32)
    for p in parts[1:]:
        out = out + p
    out = out + o_b.astype(np.float32)[None, :]
    if run_kwargs:
        return out.reshape(B, S, D), res
    return out.reshape(B, S, D)


# revision 14
# speedup vs baseline: 1.4637x; 1.0046x over previous
"""GQA attention kernel for Trainium2, sharded over 8 NeuronCores.

Problem: X (1, 4096, 1024), H=16 q-heads, KVH=4 kv-heads, head_dim=64.
Sharding: 2 q-heads + their shared kv-head per core (tensor parallel over H).
Each core computes q/k/v projections for its heads, fused flash-style
attention, and its 128-row slice of the output projection -> partial
(4096, 1024) f32, summed on host.

v2: software-pipelined single instruction stream tuned to keep the PE
array continuously busy (it only reaches the 2.4 GHz p-state after ~3us
without stalls; the v1 kernel sat at 1.2 GHz through all of attention):
  - scores(t) are emitted before PV(t-1), so exp(t-1) on the Scalar
    engine overlaps score matmuls and PV never waits on it.
  - KV projection (chunk 0), Q projection (chunk c+1) and the output
    projection (chunk c-1) are spread through the attention loop as PE
    filler work instead of serial prologue/epilogue phases.
  - softmax normalization broadcasts the reciprocal denominators with a
    GpSimd partition_broadcast (v1 round-tripped through DRAM, stalling
    the PE ~20us per chunk).

Layouts on device (per core):
  xt   : X^T            (1024 D, 4096 S)  bf16   (host pre-transposed)
  qt   : Q^T            (128 = 2 heads x 64 d, 4096 q) bf16
  kvt  : [K^T; V^T]     (128 = 64 k-d + 64 v-d, 4096 s) bf16
  kt2  : K^T duplicated into both partition halves
  v    : V natural+ones (128 s-tile, 65) x 32 tiles bf16 (col 64 == 1.0)
  St   : scores^T       (128 k, 1024 q) f32 PSUM  = Kt_tile.T @ Qt
  Pt   : exp(St/8)      (128 k, 1024 q) bf16 SBUF (ScalarE, scale folded)
  Ot   : V_aug.T @ Pt   (65, 1024) f32 PSUM; row 64 = softmax denominators
  y    : partial output (4096, 1024) f32 = (Ot/denoms).T @ o_w[rows]
"""

import sys

import numpy as np

try:
    import concourse.bass as bass
except ImportError:  # grading env may not have concourse on sys.path
    for p in ("/opt/trn_rl_repo", "/root/.axon_site/_ro/trn_rl_repo"):
        if p not in sys.path:
            sys.path.append(p)
    import concourse.bass as bass

import bass_rust
import ml_dtypes
from concourse import mybir
from concourse.bass_utils import run_bass_kernel_spmd
from concourse.masks import make_identity
from concourse.tile import TileContext

BF16 = ml_dtypes.bfloat16

B, S, D = 1, 4096, 1024
H, KVH, HD = 16, 4, 64
NCORES = 8
HPC = H // NCORES          # 2 q heads per core
DQ = HPC * HD              # 128 projected q dims per core
DKV = 2 * HD               # 128 = k head + v head dims
QC = 1024                  # attention q-chunk (2 PSUM banks per score tile)
KT = 128                   # k tile (seq positions per score tile)
NKT = S // KT              # 32
NQC = S // QC              # 4
NDC = D // 128             # 8 contraction chunks for projections
MM_N = 512                 # max matmul free dim (one PSUM bank, f32)

_COMPILED = None


def build_bass():
    nc = bass.Bass()
    fp32 = mybir.dt.float32
    bf16 = mybir.dt.bfloat16

    xt = nc.declare_dram_parameter("xt", [D, S], bf16, isOutput=False)
    qw = nc.declare_dram_parameter("qw", [D, DQ], bf16, isOutput=False)
    kvw = nc.declare_dram_parameter("kvw", [D, DKV], bf16, isOutput=False)
    ow = nc.declare_dram_parameter("ow", [DQ, D], bf16, isOutput=False)
    qb = nc.declare_dram_parameter("qb", [DQ, 1], fp32, isOutput=False)
    kvb = nc.declare_dram_parameter("kvb", [DKV, 1], fp32, isOutput=False)
    y = nc.declare_dram_parameter("y", [S, D], fp32, isOutput=True)

    with TileContext(nc) as tc:
        with (
            tc.tile_pool(name="singles", bufs=1) as singles,
            tc.tile_pool(name="pt_pool", bufs=6) as pt_pool,
            tc.tile_pool(name="nrm", bufs=2) as nrm_pool,
            tc.tile_pool(name="otcp", bufs=2) as otcp_pool,
            tc.tile_pool(name="ysb", bufs=3) as ysb_pool,
            tc.tile_pool(name="ps_st", bufs=2, space="PSUM") as ps_st,
            tc.tile_pool(name="ps_ot", bufs=2, space="PSUM") as ps_ot,
        ):
            # ---- constants / weights ----
            ident = singles.tile([128, 128], bf16)
            make_identity(nc, ident)

            # xt first: it paces the prologue (8 MB at the HBM roofline);
            # ow is only needed ~90us in, so it goes last.
            xt_sb = singles.tile([128, NDC, S], bf16)
            for j in range(S // MM_N):
                nc.sync.dma_start(
                    out=xt_sb[:, :, bass.ts(j, MM_N)],
                    in_=xt[:, :].rearrange("(c p) s -> p c s", p=128)[
                        :, :, bass.ts(j, MM_N)],
                )
            kvw_sb = singles.tile([128, NDC, DKV], bf16)
            nc.sync.dma_start(
                out=kvw_sb, in_=kvw[:, :].rearrange("(c p) m -> p c m", p=128)
            )
            qw_sb = singles.tile([128, NDC, DQ], bf16)
            nc.sync.dma_start(
                out=qw_sb, in_=qw[:, :].rearrange("(c p) m -> p c m", p=128)
            )
            qb_sb = singles.tile([DQ, 1], fp32)
            nc.sync.dma_start(out=qb_sb, in_=qb[:, :])
            kvb_sb = singles.tile([DKV, 1], fp32)
            nc.sync.dma_start(out=kvb_sb, in_=kvb[:, :])
            ow_sb = singles.tile([DQ, D], bf16)
            nc.sync.dma_start(out=ow_sb, in_=ow[:, :])

            qt_sb = singles.tile([DQ, S], bf16)
            kvt_sb = singles.tile([DKV, S], bf16)
            kt2_sb = singles.tile([DKV, S], bf16)
            v_sb = singles.tile([128, NKT, HD + 1], bf16)
            nc.vector.memset(v_sb, 1.0)
            ot_full = singles.tile([DQ, S], bf16)
            ones_col = singles.tile([1, HD], bf16)
            nc.vector.memset(ones_col, 1.0)

            exp = mybir.ActivationFunctionType.Exp

            def proj_slice(dst, w_sb, b_sb, j):
                ps = ps_st.tile([128, MM_N], fp32, tag="st")
                for c2 in range(NDC):
                    nc.tensor.matmul(
                        ps, w_sb[:, c2, :], xt_sb[:, c2, bass.ts(j, MM_N)],
                        start=(c2 == 0), stop=(c2 == NDC - 1),
                    )
                nc.vector.tensor_scalar_add(
                    dst[:, bass.ts(j, MM_N)], ps, b_sb[:, 0:1]
                )

            def kv_slice(j):
                proj_slice(kvt_sb, kvw_sb, kvb_sb, j)
                nc.sync.dma_start(
                    out=kt2_sb[0:HD, bass.ts(j, MM_N)],
                    in_=kvt_sb[0:HD, bass.ts(j, MM_N)],
                )
                nc.sync.dma_start(
                    out=kt2_sb[HD:DKV, bass.ts(j, MM_N)],
                    in_=kvt_sb[0:HD, bass.ts(j, MM_N)],
                )
                for tt in range(4 * j, 4 * j + 4):
                    pvt = ps_st.tile([128, HD], bf16, tag="st")
                    nc.tensor.transpose(
                        pvt, kvt_sb[HD:DKV, bass.ts(tt, KT)],
                        ident[HD:DKV, HD:DKV],
                    )
                    nc.vector.tensor_copy(v_sb[:, tt, 0:HD], pvt)

            def emit_scores(c, t):
                res = []
                for h in range(HPC):
                    st = ps_st.tile([128, QC], fp32, tag="st")
                    for u in range(QC // MM_N):
                        nc.tensor.matmul(
                            st[:, bass.ts(u, MM_N)],
                            kt2_sb[h * HD:(h + 1) * HD, bass.ts(t, KT)],
                            qt_sb[h * HD:(h + 1) * HD,
                                  c * QC + u * MM_N:c * QC + (u + 1) * MM_N],
                            start=True, stop=True,
                        )
                    pt = pt_pool.tile([128, QC], bf16, tag="pt")
                    nc.scalar.activation(pt, st, exp, scale=1.0 / 8.0)
                    res.append(pt)
                return res

            ots = {}

            def emit_pv(pc, pt_, ppts):
                if pt_ == 0:
                    ot_a = ps_ot.tile([HD + 1, QC], fp32, tag="ot")
                    ot_b = ps_ot.tile([HD + 1, QC], fp32, tag="ot")
                    ots[pc] = (ot_a, ot_b)
                for o, pp in zip(ots[pc], ppts):
                    for u in range(QC // MM_N):
                        nc.tensor.matmul(
                            o[:, bass.ts(u, MM_N)], v_sb[:, pt_, :],
                            pp[:, bass.ts(u, MM_N)],
                            start=(pt_ == 0), stop=(pt_ == NKT - 1),
                        )

            otcps = {}

            def emit_otcp(pc):
                # Free the PSUM accumulators fast (one DVE copy each) so the
                # next chunk's PV matmuls aren't blocked behind the slow
                # normalization chain; the divide happens lazily off ot_cp.
                cp_a = otcp_pool.tile([HD + 1, QC], fp32, tag="otcp")
                cp_b = otcp_pool.tile([HD + 1, QC], fp32, tag="otcp")
                nc.vector.tensor_copy(cp_a, ots[pc][0])
                nc.vector.tensor_copy(cp_b, ots[pc][1])
                otcps[pc] = (cp_a, cp_b)
                del ots[pc]

            def emit_norm_piece(pc, h, u, use_act=False):
                # ot rows 0..63 / row 64 (denominators): reciprocal, round to
                # bf16, broadcast across the 64 hd partitions with a rank-1
                # PE matmul (ones column), multiply on DVE. The DVE IEEE
                # reciprocal costs ~3.3us per 512 elems, so in steady state it
                # runs off the critical path; in the epilogue (use_act=True)
                # the idle Scalar engine computes 1/d = exp(-ln d) instead.
                o = otcps[pc][h]
                usl = bass.ts(u, MM_N)
                rsb = nrm_pool.tile([1, MM_N], bf16, tag="rsb")
                if use_act:
                    rs = nrm_pool.tile([1, MM_N], fp32, tag="rs")
                    nc.scalar.activation(
                        rs, o[HD:HD + 1, usl],
                        mybir.ActivationFunctionType.Ln,
                    )
                    nc.scalar.activation(rsb, rs, exp, scale=-1.0)
                else:
                    rs = nrm_pool.tile([1, MM_N], fp32, tag="rs")
                    nc.vector.reciprocal(rs, o[HD:HD + 1, usl])
                    nc.vector.tensor_copy(rsb, rs)
                bc_ps = ps_st.tile([HD, MM_N], fp32, tag="st")
                nc.tensor.matmul(bc_ps, ones_col, rsb, start=True, stop=True)
                nc.vector.tensor_mul(
                    ot_full[h * HD:(h + 1) * HD,
                            pc * QC + u * MM_N:pc * QC + (u + 1) * MM_N],
                    o[0:HD, usl], bc_ps,
                )

            def outproj_piece(jq):
                for u2 in range(D // MM_N):
                    yp = ps_st.tile([128, MM_N], fp32, tag="st")
                    nc.tensor.matmul(
                        yp, ot_full[:, bass.ts(jq, 128)],
                        ow_sb[:, bass.ts(u2, MM_N)],
                        start=True, stop=True,
                    )
                    ysb = ysb_pool.tile([128, MM_N], fp32, tag="ysb")
                    nc.vector.tensor_copy(ysb, yp)
                    nc.sync.dma_start(
                        out=y[:, :][bass.ts(jq, 128), bass.ts(u2, MM_N)],
                        in_=ysb,
                    )

            # ---- prologue: first KV slice, V tiles 0-3, Q proj chunk 0 ----
            kv_slice(0)
            proj_slice(qt_sb, qw_sb, qb_sb, 0)
            proj_slice(qt_sb, qw_sb, qb_sb, 1)

            # ---- main software-pipelined loop ----
            # norm pieces (h, u) for the previous chunk run at steps 1-4;
            # outproj pieces follow once their ot_full half is normalized.
            NORM_STEPS = {1: (0, 0), 2: (1, 0), 3: (0, 1), 4: (1, 1)}
            OUTPROJ_STEPS = (6, 8, 10, 13, 16, 19, 22, 25)
            pending_outproj = []
            pending_norm = []
            prev = None
            for s_ in range(NQC * NKT):
                c, t = divmod(s_, NKT)
                pts = emit_scores(c, t)
                if prev is not None:
                    (pc, pt_), ppts = prev
                    emit_pv(pc, pt_, ppts)
                    if pt_ == NKT - 1:
                        emit_otcp(pc)
                        pending_norm = [
                            (pc, h, u) for u in range(2) for h in range(2)
                        ]
                        pending_outproj.extend(
                            range(pc * (QC // 128), (pc + 1) * (QC // 128))
                        )
                # ---- PE filler work (keeps the systolic array saturated) ----
                if pending_norm and t in NORM_STEPS:
                    emit_norm_piece(*pending_norm.pop(0))
                if c == 0 and t >= 2 and (t - 2) % 4 == 0:
                    j = (t - 2) // 4 + 1
                    if j < NDC:
                        kv_slice(j)
                if c + 1 < NQC:
                    if c == 0:
                        if t == 27:
                            proj_slice(qt_sb, qw_sb, qb_sb, 2)
                        elif t == 29:
                            proj_slice(qt_sb, qw_sb, qb_sb, 3)
                    else:
                        if t == 6:
                            proj_slice(qt_sb, qw_sb, qb_sb, 2 * (c + 1))
                        elif t == 22:
                            proj_slice(qt_sb, qw_sb, qb_sb, 2 * (c + 1) + 1)
                if pending_outproj and t in OUTPROJ_STEPS:
                    outproj_piece(pending_outproj.pop(0))
                prev = ((c, t), pts)

            # ---- epilogue: last chunk's PV tail, norm via idle ScalarE ----
            (pc, pt_), ppts = prev
            emit_pv(pc, pt_, ppts)
            emit_otcp(pc)
            for u in range(2):
                for h in range(2):
                    emit_norm_piece(pc, h, u, use_act=True)
            pending_outproj.extend(
                range(pc * (QC // 128), (pc + 1) * (QC // 128))
            )
            for jq in pending_outproj:
                outproj_piece(jq)
    _split_multi_waits(nc)
    return nc


def _split_multi_waits(nc):
    """This toolchain's walrus accepts at most one sync-wait per datapath
    instruction; move extra waits onto same-engine NoOps placed just before."""
    k = 0
    for f in nc.m.functions:
        for blk in f.blocks:
            out = []
            for inst in blk.instructions:
                si = getattr(inst, "sync_info", None)
                ow_ = list(si.on_wait) if (si and si.on_wait) else []
                if len(ow_) > 1:
                    for w in ow_[:-1]:
                        k += 1
                        nop = bass_rust.InstNoOp(
                            name=f"I-wsplit-{k}", ins=[], outs=[]
                        )
                        nop.engine = inst.engine
                        nop.sync_info = mybir.SyncInfo(
                            on_wait=[w], on_update=[]
                        )
                        out.append(nop)
                    inst.sync_info = mybir.SyncInfo(
                        on_wait=[ow_[-1]], on_update=list(si.on_update or [])
                    )
                out.append(inst)
            blk.instructions = out


def _prep_inputs(X, q_w, q_b, k_w, k_b, v_w, v_b, o_w):
    Xt = np.ascontiguousarray(X.reshape(S, D).T).astype(BF16)
    in_maps = []
    for c in range(NCORES):
        kv = c // (NCORES // KVH)
        qs = slice(c * DQ, (c + 1) * DQ)
        ks = slice(kv * HD, (kv + 1) * HD)
        in_maps.append({
            "xt": Xt,
            "qw": np.ascontiguousarray(q_w[:, qs]).astype(BF16),
            "kvw": np.ascontiguousarray(
                np.concatenate([k_w[:, ks], v_w[:, ks]], axis=1)).astype(BF16),
            "ow": np.ascontiguousarray(o_w[qs, :]).astype(BF16),
            "qb": np.ascontiguousarray(q_b[qs]).reshape(DQ, 1).astype(
                np.float32),
            "kvb": np.ascontiguousarray(
                np.concatenate([k_b[ks], v_b[ks]])).reshape(DKV, 1).astype(
                np.float32),
        })
    return in_maps


def kernel(X, q_w, q_b, k_w, k_b, v_w, v_b, o_w, o_b, **run_kwargs):
    global _COMPILED
    if _COMPILED is None:
        _COMPILED = build_bass()
    in_maps = _prep_inputs(X, q_w, q_b, k_w, k_b, v_w, v_b, o_w)
    res = run_bass_kernel_spmd(
        _COMPILED, in_maps, list(range(NCORES)), **run_kwargs
    )
    parts = [r["y"] for r in res.results]
    out = parts[0].astype(np.float32)
    for p in parts[1:]:
        out = out + p
    out = out + o_b.astype(np.float32)[None, :]
    if run_kwargs:
        return out.reshape(B, S, D), res
    return out.reshape(B, S, D)


# revision 18
# speedup vs baseline: 1.5632x; 1.0680x over previous
"""GQA attention kernel for Trainium2, sharded over 8 NeuronCores.

Problem: X (1, 4096, 1024), H=16 q-heads, KVH=4 kv-heads, head_dim=64.
Sharding: 2 q-heads + their shared kv-head per core (tensor parallel over H).
Each core computes q/k/v projections for its heads, fused flash-style
attention, and its 128-row slice of the output projection -> partial
(4096, 1024) f32, summed on host.

v2: software-pipelined single instruction stream tuned to keep the PE
array continuously busy (it only reaches the 2.4 GHz p-state after ~3us
without stalls; the v1 kernel sat at 1.2 GHz through all of attention):
  - scores(t) are emitted before PV(t-1), so exp(t-1) on the Scalar
    engine overlaps score matmuls and PV never waits on it.
  - KV projection (chunk 0), Q projection (chunk c+1) and the output
    projection (chunk c-1) are spread through the attention loop as PE
    filler work instead of serial prologue/epilogue phases.
  - softmax normalization broadcasts the reciprocal denominators with a
    GpSimd partition_broadcast (v1 round-tripped through DRAM, stalling
    the PE ~20us per chunk).

Layouts on device (per core):
  xt   : X^T            (1024 D, 4096 S)  bf16   (host pre-transposed)
  qt   : Q^T            (128 = 2 heads x 64 d, 4096 q) bf16
  kvt  : [K^T; V^T]     (128 = 64 k-d + 64 v-d, 4096 s) bf16
  kt2  : K^T duplicated into both partition halves
  v    : V natural+ones (128 s-tile, 65) x 32 tiles bf16 (col 64 == 1.0)
  St   : scores^T       (128 k, 1024 q) f32 PSUM  = Kt_tile.T @ Qt
  Pt   : exp(St/8)      (128 k, 1024 q) bf16 SBUF (ScalarE, scale folded)
  Ot   : V_aug.T @ Pt   (65, 1024) f32 PSUM; row 64 = softmax denominators
  y    : partial output (4096, 1024) f32 = (Ot/denoms).T @ o_w[rows]
"""

import sys

import numpy as np

try:
    import concourse.bass as bass
except ImportError:  # grading env may not have concourse on sys.path
    for p in ("/opt/trn_rl_repo", "/root/.axon_site/_ro/trn_rl_repo"):
        if p not in sys.path:
            sys.path.append(p)
    import concourse.bass as bass

import bass_rust
import ml_dtypes
from concourse import mybir
from concourse.bass_utils import run_bass_kernel_spmd
from concourse.masks import make_identity
from concourse.tile import TileContext

BF16 = ml_dtypes.bfloat16

B, S, D = 1, 4096, 1024
H, KVH, HD = 16, 4, 64
NCORES = 8
HPC = H // NCORES          # 2 q heads per core
DQ = HPC * HD              # 128 projected q dims per core
DKV = 2 * HD               # 128 = k head + v head dims
QC = 1024                  # attention q-chunk (2 PSUM banks per score tile)
KT = 128                   # k tile (seq positions per score tile)
NKT = S // KT              # 32
NQC = S // QC              # 4
NDC = D // 128             # 8 contraction chunks for projections
MM_N = 512                 # max matmul free dim (one PSUM bank, f32)

_COMPILED = None


def build_bass():
    nc = bass.Bass()
    fp32 = mybir.dt.float32
    bf16 = mybir.dt.bfloat16

    xt = nc.declare_dram_parameter("xt", [D, S], bf16, isOutput=False)
    qw = nc.declare_dram_parameter("qw", [D, DQ], bf16, isOutput=False)
    kvw = nc.declare_dram_parameter("kvw", [D, DKV], bf16, isOutput=False)
    ow = nc.declare_dram_parameter("ow", [DQ, D], bf16, isOutput=False)
    qb = nc.declare_dram_parameter("qb", [DQ, 1], fp32, isOutput=False)
    kvb = nc.declare_dram_parameter("kvb", [DKV, 1], fp32, isOutput=False)
    y = nc.declare_dram_parameter("y", [S, D], fp32, isOutput=True)

    with TileContext(nc) as tc:
        with (
            tc.tile_pool(name="singles", bufs=1) as singles,
            tc.tile_pool(name="pt_pool", bufs=8) as pt_pool,
            tc.tile_pool(name="nrm", bufs=2) as nrm_pool,
            tc.tile_pool(name="otcp", bufs=2) as otcp_pool,
            tc.tile_pool(name="ysb", bufs=3) as ysb_pool,
            tc.tile_pool(name="ps_st", bufs=2, space="PSUM") as ps_st,
            tc.tile_pool(name="ps_ot", bufs=2, space="PSUM") as ps_ot,
        ):
            # ---- constants / weights ----
            ident = singles.tile([128, 128], bf16)
            make_identity(nc, ident)

            # DMA order matters: the prologue needs xt slices 0-1 and the
            # q/kv weights first; the remaining xt slices pace the chunk-0
            # kv fillers; ow is only needed ~100us in, so it goes last.
            xt_sb = singles.tile([128, NDC, S], bf16)
            xt_re = xt[:, :].rearrange("(c p) s -> p c s", p=128)
            for j in range(2):
                nc.sync.dma_start(
                    out=xt_sb[:, :, bass.ts(j, MM_N)],
                    in_=xt_re[:, :, bass.ts(j, MM_N)],
                )
            kvw_sb = singles.tile([128, NDC, DKV], bf16)
            nc.sync.dma_start(
                out=kvw_sb, in_=kvw[:, :].rearrange("(c p) m -> p c m", p=128)
            )
            qw_sb = singles.tile([128, NDC, DQ], bf16)
            nc.sync.dma_start(
                out=qw_sb, in_=qw[:, :].rearrange("(c p) m -> p c m", p=128)
            )
            qb_sb = singles.tile([DQ, 1], fp32)
            nc.sync.dma_start(out=qb_sb, in_=qb[:, :])
            kvb_sb = singles.tile([DKV, 1], fp32)
            nc.sync.dma_start(out=kvb_sb, in_=kvb[:, :])
            for j in range(2, S // MM_N):
                nc.sync.dma_start(
                    out=xt_sb[:, :, bass.ts(j, MM_N)],
                    in_=xt_re[:, :, bass.ts(j, MM_N)],
                )
            ow_sb = singles.tile([DQ, D], bf16)
            nc.sync.dma_start(out=ow_sb, in_=ow[:, :])

            qt_sb = singles.tile([DQ, S], bf16)
            kvt_sb = singles.tile([DKV, S], bf16)
            kt2_sb = singles.tile([DKV, S], bf16)
            v_sb = singles.tile([128, NKT, HD + 1], bf16)
            nc.vector.memset(v_sb, 1.0)
            ot_full = singles.tile([DQ, S], bf16)
            ones_col = singles.tile([1, HD], bf16)
            nc.vector.memset(ones_col, 1.0)

            exp = mybir.ActivationFunctionType.Exp

            def proj_slice(dst, w_sb, b_sb, j):
                ps = ps_st.tile([128, MM_N], fp32, tag="st")
                for c2 in range(NDC):
                    nc.tensor.matmul(
                        ps, w_sb[:, c2, :], xt_sb[:, c2, bass.ts(j, MM_N)],
                        start=(c2 == 0), stop=(c2 == NDC - 1),
                    )
                nc.vector.tensor_scalar_add(
                    dst[:, bass.ts(j, MM_N)], ps, b_sb[:, 0:1]
                )

            def kv_slice(j):
                proj_slice(kvt_sb, kvw_sb, kvb_sb, j)
                nc.sync.dma_start(
                    out=kt2_sb[0:HD, bass.ts(j, MM_N)],
                    in_=kvt_sb[0:HD, bass.ts(j, MM_N)],
                )
                nc.sync.dma_start(
                    out=kt2_sb[HD:DKV, bass.ts(j, MM_N)],
                    in_=kvt_sb[0:HD, bass.ts(j, MM_N)],
                )
                for tt in range(4 * j, 4 * j + 4):
                    pvt = ps_st.tile([128, HD], bf16, tag="st")
                    nc.tensor.transpose(
                        pvt, kvt_sb[HD:DKV, bass.ts(tt, KT)],
                        ident[HD:DKV, HD:DKV],
                    )
                    nc.vector.tensor_copy(v_sb[:, tt, 0:HD], pvt)

            def emit_scores(c, t):
                res = []
                for h in range(HPC):
                    st = ps_st.tile([128, QC], fp32, tag="st")
                    for u in range(QC // MM_N):
                        nc.tensor.matmul(
                            st[:, bass.ts(u, MM_N)],
                            kt2_sb[h * HD:(h + 1) * HD, bass.ts(t, KT)],
                            qt_sb[h * HD:(h + 1) * HD,
                                  c * QC + u * MM_N:c * QC + (u + 1) * MM_N],
                            start=True, stop=True,
                        )
                    pt = pt_pool.tile([128, QC], bf16, tag="pt")
                    nc.scalar.activation(pt, st, exp, scale=1.0 / 8.0)
                    res.append(pt)
                return res

            ots = {}

            def emit_pv(pc, pt_, ppts):
                if pt_ == 0:
                    ot_a = ps_ot.tile([HD + 1, QC], fp32, tag="ot")
                    ot_b = ps_ot.tile([HD + 1, QC], fp32, tag="ot")
                    ots[pc] = (ot_a, ot_b)
                for o, pp in zip(ots[pc], ppts):
                    for u in range(QC // MM_N):
                        nc.tensor.matmul(
                            o[:, bass.ts(u, MM_N)], v_sb[:, pt_, :],
                            pp[:, bass.ts(u, MM_N)],
                            start=(pt_ == 0), stop=(pt_ == NKT - 1),
                        )

            otcps = {}

            def emit_otcp(pc):
                # Free the PSUM accumulators fast (one DVE copy each) so the
                # next chunk's PV matmuls aren't blocked behind the slow
                # normalization chain; the divide happens lazily off ot_cp.
                cp_a = otcp_pool.tile([HD + 1, QC], fp32, tag="otcp")
                cp_b = otcp_pool.tile([HD + 1, QC], fp32, tag="otcp")
                nc.vector.tensor_copy(cp_a, ots[pc][0])
                nc.scalar.activation(  # ACT is idle here; halves the wait
                    cp_b, ots[pc][1], mybir.ActivationFunctionType.Copy
                )
                otcps[pc] = (cp_a, cp_b)
                del ots[pc]

            def emit_norm_piece(pc, h, u, use_act=False):
                # ot rows 0..63 / row 64 (denominators): reciprocal, round to
                # bf16, broadcast across the 64 hd partitions with a rank-1
                # PE matmul (ones column), multiply on DVE. The DVE IEEE
                # reciprocal costs ~3.3us per 512 elems, so in steady state it
                # runs off the critical path; in the epilogue (use_act=True)
                # the idle Scalar engine computes 1/d = exp(-ln d) instead.
                o = otcps[pc][h]
                usl = bass.ts(u, MM_N)
                rsb = nrm_pool.tile([1, MM_N], bf16, tag="rsb")
                if use_act:
                    rs = nrm_pool.tile([1, MM_N], fp32, tag="rs")
                    nc.scalar.activation(
                        rs, o[HD:HD + 1, usl],
                        mybir.ActivationFunctionType.Ln,
                    )
                    nc.scalar.activation(rsb, rs, exp, scale=-1.0)
                else:
                    rs = nrm_pool.tile([1, MM_N], fp32, tag="rs")
                    nc.vector.reciprocal(rs, o[HD:HD + 1, usl])
                    nc.vector.tensor_copy(rsb, rs)
                bc_ps = ps_st.tile([HD, MM_N], fp32, tag="st")
                nc.tensor.matmul(bc_ps, ones_col, rsb, start=True, stop=True)
                nc.vector.tensor_mul(
                    ot_full[h * HD:(h + 1) * HD,
                            pc * QC + u * MM_N:pc * QC + (u + 1) * MM_N],
                    o[0:HD, usl], bc_ps,
                )

            def outproj_piece(jq):
                for u2 in range(D // MM_N):
                    yp = ps_st.tile([128, MM_N], fp32, tag="st")
                    nc.tensor.matmul(
                        yp, ot_full[:, bass.ts(jq, 128)],
                        ow_sb[:, bass.ts(u2, MM_N)],
                        start=True, stop=True,
                    )
                    ysb = ysb_pool.tile([128, MM_N], fp32, tag="ysb")
                    nc.vector.tensor_copy(ysb, yp)
                    nc.sync.dma_start(
                        out=y[:, :][bass.ts(jq, 128), bass.ts(u2, MM_N)],
                        in_=ysb,
                    )

            # ---- prologue: first KV slice, V tiles 0-3, Q proj chunk 0 ----
            kv_slice(0)
            proj_slice(qt_sb, qw_sb, qb_sb, 0)
            proj_slice(qt_sb, qw_sb, qb_sb, 1)

            # ---- main software-pipelined loop ----
            # norm pieces (h, u) for the previous chunk run at steps 5-17,
            # spaced so each DVE reciprocal (~3.3us) completes before its
            # PE broadcast matmul is enqueued (no head-of-line blocking);
            # outproj pieces follow once their ot_full half is normalized.
            NORM_STEPS = (5, 9, 13, 17)
            OUTPROJ_STEPS = (11, 13, 15, 17, 19, 21, 23, 25)
            pending_outproj = []
            pending_norm = []
            prev = None
            for s_ in range(NQC * NKT):
                c, t = divmod(s_, NKT)
                pts = emit_scores(c, t)
                if prev is not None:
                    (pc, pt_), ppts = prev
                    emit_pv(pc, pt_, ppts)
                    if pt_ == NKT - 1:
                        emit_otcp(pc)
                        pending_norm = [
                            (pc, h, u) for u in range(2) for h in range(2)
                        ]
                        pending_outproj.extend(
                            range(pc * (QC // 128), (pc + 1) * (QC // 128))
                        )
                # ---- PE filler work (keeps the systolic array saturated) ----
                if pending_norm and t in NORM_STEPS:
                    emit_norm_piece(*pending_norm.pop(0))
                if c == 0 and t >= 2 and (t - 2) % 4 == 0:
                    j = (t - 2) // 4 + 1
                    if j < NDC:
                        kv_slice(j)
                if c + 1 < NQC:
                    if c == 0:
                        if t == 27:
                            proj_slice(qt_sb, qw_sb, qb_sb, 2)
                        elif t == 29:
                            proj_slice(qt_sb, qw_sb, qb_sb, 3)
                    else:
                        if t == 6:
                            proj_slice(qt_sb, qw_sb, qb_sb, 2 * (c + 1))
                        elif t == 22:
                            proj_slice(qt_sb, qw_sb, qb_sb, 2 * (c + 1) + 1)
                if pending_outproj and t in OUTPROJ_STEPS:
                    outproj_piece(pending_outproj.pop(0))
                prev = ((c, t), pts)

            # ---- epilogue: last chunk's PV tail, norm via idle ScalarE ----
            (pc, pt_), ppts = prev
            emit_pv(pc, pt_, ppts)
            emit_otcp(pc)
            for u in range(2):
                for h in range(2):
                    emit_norm_piece(pc, h, u, use_act=True)
            pending_outproj.extend(
                range(pc * (QC // 128), (pc + 1) * (QC // 128))
            )
            for jq in pending_outproj:
                outproj_piece(jq)
    _split_multi_waits(nc)
    return nc


def _split_multi_waits(nc):
    """This toolchain's walrus accepts at most one sync-wait per datapath
    instruction; move extra waits onto same-engine NoOps placed just before."""
    k = 0
    for f in nc.m.functions:
        for blk in f.blocks:
            out = []
            for inst in blk.instructions:
                si = getattr(inst, "sync_info", None)
                ow_ = list(si.on_wait) if (si and si.on_wait) else []
                if len(ow_) > 1:
                    for w in ow_[:-1]:
                        k += 1
                        nop = bass_rust.InstNoOp(
                            name=f"I-wsplit-{k}", ins=[], outs=[]
                        )
                        nop.engine = inst.engine
                        nop.sync_info = mybir.SyncInfo(
                            on_wait=[w], on_update=[]
                        )
                        out.append(nop)
                    inst.sync_info = mybir.SyncInfo(
                        on_wait=[ow_[-1]], on_update=list(si.on_update or [])
                    )
                out.append(inst)
            blk.instructions = out


def _prep_inputs(X, q_w, q_b, k_w, k_b, v_w, v_b, o_w):
    Xt = np.ascontiguousarray(X.reshape(S, D).T).astype(BF16)
    in_maps = []
    for c in range(NCORES):
        kv = c // (NCORES // KVH)
        qs = slice(c * DQ, (c + 1) * DQ)
        ks = slice(kv * HD, (kv + 1) * HD)
        in_maps.append({
            "xt": Xt,
            "qw": np.ascontiguousarray(q_w[:, qs]).astype(BF16),
            "kvw": np.ascontiguousarray(
                np.concatenate([k_w[:, ks], v_w[:, ks]], axis=1)).astype(BF16),
            "ow": np.ascontiguousarray(o_w[qs, :]).astype(BF16),
            "qb": np.ascontiguousarray(q_b[qs]).reshape(DQ, 1).astype(
                np.float32),
            "kvb": np.ascontiguousarray(
                np.concatenate([k_b[ks], v_b[ks]])).reshape(DKV, 1).astype(
                np.float32),
        })
    return in_maps


def kernel(X, q_w, q_b, k_w, k_b, v_w, v_b, o_w, o_b, **run_kwargs):
    global _COMPILED
    if _COMPILED is None:
        _COMPILED = build_bass()
    in_maps = _prep_inputs(X, q_w, q_b, k_w, k_b, v_w, v_b, o_w)
    res = run_bass_kernel_spmd(
        _COMPILED, in_maps, list(range(NCORES)), **run_kwargs
    )
    parts = [r["y"] for r in res.results]
    out = parts[0].astype(np.float32)
    for p in parts[1:]:
        out = out + p
    out = out + o_b.astype(np.float32)[None, :]
    if run_kwargs:
        return out.reshape(B, S, D), res
    return out.reshape(B, S, D)


# revision 21
# speedup vs baseline: 1.6303x; 1.0430x over previous
"""GQA attention kernel for Trainium2, sharded over 8 NeuronCores.

Problem: X (1, 4096, 1024), H=16 q-heads, KVH=4 kv-heads, head_dim=64.
Sharding: 2 q-heads + their shared kv-head per core (tensor parallel over H).
Each core computes q/k/v projections for its heads, fused flash-style
attention, and its 128-row slice of the output projection -> partial
(4096, 1024) f32, summed on host.

v2: software-pipelined single instruction stream tuned to keep the PE
array continuously busy (it only reaches the 2.4 GHz p-state after ~3us
without stalls; the v1 kernel sat at 1.2 GHz through all of attention):
  - scores(t) are emitted before PV(t-1), so exp(t-1) on the Scalar
    engine overlaps score matmuls and PV never waits on it.
  - KV projection (chunk 0), Q projection (chunk c+1) and the output
    projection (chunk c-1) are spread through the attention loop as PE
    filler work instead of serial prologue/epilogue phases.
  - softmax normalization broadcasts the reciprocal denominators with a
    GpSimd partition_broadcast (v1 round-tripped through DRAM, stalling
    the PE ~20us per chunk).

Layouts on device (per core):
  xt   : X^T            (1024 D, 4096 S)  bf16   (host pre-transposed)
  qt   : Q^T            (128 = 2 heads x 64 d, 4096 q) bf16
  kvt  : [K^T; V^T]     (128 = 64 k-d + 64 v-d, 4096 s) bf16
  kt2  : K^T duplicated into both partition halves
  v    : V natural+ones (128 s-tile, 65) x 32 tiles bf16 (col 64 == 1.0)
  St   : scores^T       (128 k, 1024 q) f32 PSUM  = Kt_tile.T @ Qt
  Pt   : exp(St/8)      (128 k, 1024 q) bf16 SBUF (ScalarE, scale folded)
  Ot   : V_aug.T @ Pt   (65, 1024) f32 PSUM; row 64 = softmax denominators
  y    : partial output (4096, 1024) f32 = (Ot/denoms).T @ o_w[rows]
"""

import sys

import numpy as np

try:
    import concourse.bass as bass
except ImportError:  # grading env may not have concourse on sys.path
    for p in ("/opt/trn_rl_repo", "/root/.axon_site/_ro/trn_rl_repo"):
        if p not in sys.path:
            sys.path.append(p)
    import concourse.bass as bass

import bass_rust
import ml_dtypes
from concourse import mybir
from concourse.bass_utils import run_bass_kernel_spmd
from concourse.masks import make_identity
from concourse.tile import TileContext

BF16 = ml_dtypes.bfloat16

B, S, D = 1, 4096, 1024
H, KVH, HD = 16, 4, 64
NCORES = 8
HPC = H // NCORES          # 2 q heads per core
DQ = HPC * HD              # 128 projected q dims per core
DKV = 2 * HD               # 128 = k head + v head dims
QC = 1024                  # attention q-chunk (2 PSUM banks per score tile)
KT = 128                   # k tile (seq positions per score tile)
NKT = S // KT              # 32
NQC = S // QC              # 4
NDC = D // 128             # 8 contraction chunks for projections
MM_N = 512                 # max matmul free dim (one PSUM bank, f32)

_COMPILED = None


def build_bass():
    nc = bass.Bass()
    fp32 = mybir.dt.float32
    bf16 = mybir.dt.bfloat16

    xt = nc.declare_dram_parameter("xt", [D, S], bf16, isOutput=False)
    qw = nc.declare_dram_parameter("qw", [D, DQ], bf16, isOutput=False)
    kvw = nc.declare_dram_parameter("kvw", [D, DKV], bf16, isOutput=False)
    ow = nc.declare_dram_parameter("ow", [DQ, D], bf16, isOutput=False)
    qb = nc.declare_dram_parameter("qb", [DQ, 1], fp32, isOutput=False)
    kvb = nc.declare_dram_parameter("kvb", [DKV, 1], fp32, isOutput=False)
    y = nc.declare_dram_parameter("y", [S, D], fp32, isOutput=True)

    with TileContext(nc) as tc:
        with (
            tc.tile_pool(name="singles", bufs=1) as singles,
            tc.tile_pool(name="pt_pool", bufs=8) as pt_pool,
            tc.tile_pool(name="nrm", bufs=2) as nrm_pool,
            tc.tile_pool(name="otcp", bufs=2) as otcp_pool,
            tc.tile_pool(name="ysb", bufs=3) as ysb_pool,
            tc.tile_pool(name="ps_st", bufs=2, space="PSUM") as ps_st,
            tc.tile_pool(name="ps_ot", bufs=2, space="PSUM") as ps_ot,
        ):
            # ---- constants / weights ----
            ident = singles.tile([128, 128], bf16)
            make_identity(nc, ident)

            # DMA order matters: the prologue needs xt slices 0-1 and the
            # q/kv weights first; the remaining xt slices pace the chunk-0
            # kv fillers; ow is only needed ~100us in, so it goes last.
            xt_sb = singles.tile([128, NDC, S], bf16)
            xt_re = xt[:, :].rearrange("(c p) s -> p c s", p=128)
            for j in range(2):
                nc.sync.dma_start(
                    out=xt_sb[:, :, bass.ts(j, MM_N)],
                    in_=xt_re[:, :, bass.ts(j, MM_N)],
                )
            kvw_sb = singles.tile([128, NDC, DKV], bf16)
            nc.sync.dma_start(
                out=kvw_sb, in_=kvw[:, :].rearrange("(c p) m -> p c m", p=128)
            )
            qw_sb = singles.tile([128, NDC, DQ], bf16)
            nc.sync.dma_start(
                out=qw_sb, in_=qw[:, :].rearrange("(c p) m -> p c m", p=128)
            )
            qb_sb = singles.tile([DQ, 1], fp32)
            nc.sync.dma_start(out=qb_sb, in_=qb[:, :])
            kvb_sb = singles.tile([DKV, 1], fp32)
            nc.sync.dma_start(out=kvb_sb, in_=kvb[:, :])
            for j in range(2, S // MM_N):
                nc.sync.dma_start(
                    out=xt_sb[:, :, bass.ts(j, MM_N)],
                    in_=xt_re[:, :, bass.ts(j, MM_N)],
                )
            ow_sb = singles.tile([DQ, D], bf16)
            nc.sync.dma_start(out=ow_sb, in_=ow[:, :])

            qt_sb = singles.tile([DQ, S], bf16)
            kvt_sb = singles.tile([DKV, S], bf16)
            kt2_sb = singles.tile([DKV, S], bf16)
            v_sb = singles.tile([128, NKT, HD + 1], bf16)
            nc.vector.memset(v_sb, 1.0)
            ot_full = singles.tile([DQ, S], bf16)
            ones_col = singles.tile([1, HD], bf16)
            nc.vector.memset(ones_col, 1.0)

            exp = mybir.ActivationFunctionType.Exp

            def proj_slice(dst, w_sb, b_sb, j):
                ps = ps_st.tile([128, MM_N], fp32, tag="st")
                for c2 in range(NDC):
                    nc.tensor.matmul(
                        ps, w_sb[:, c2, :], xt_sb[:, c2, bass.ts(j, MM_N)],
                        start=(c2 == 0), stop=(c2 == NDC - 1),
                    )
                nc.vector.tensor_scalar_add(
                    dst[:, bass.ts(j, MM_N)], ps, b_sb[:, 0:1]
                )

            def kv_slice(j):
                proj_slice(kvt_sb, kvw_sb, kvb_sb, j)
                nc.sync.dma_start(
                    out=kt2_sb[0:HD, bass.ts(j, MM_N)],
                    in_=kvt_sb[0:HD, bass.ts(j, MM_N)],
                )
                nc.sync.dma_start(
                    out=kt2_sb[HD:DKV, bass.ts(j, MM_N)],
                    in_=kvt_sb[0:HD, bass.ts(j, MM_N)],
                )
                for tt in range(4 * j, 4 * j + 4):
                    pvt = ps_st.tile([128, HD], bf16, tag="st")
                    nc.tensor.transpose(
                        pvt, kvt_sb[HD:DKV, bass.ts(tt, KT)],
                        ident[HD:DKV, HD:DKV],
                    )
                    nc.vector.tensor_copy(v_sb[:, tt, 0:HD], pvt)

            def emit_scores(c, t):
                res = []
                for h in range(HPC):
                    st = ps_st.tile([128, QC], fp32, tag="st")
                    for u in range(QC // MM_N):
                        nc.tensor.matmul(
                            st[:, bass.ts(u, MM_N)],
                            kt2_sb[h * HD:(h + 1) * HD, bass.ts(t, KT)],
                            qt_sb[h * HD:(h + 1) * HD,
                                  c * QC + u * MM_N:c * QC + (u + 1) * MM_N],
                            start=True, stop=True,
                        )
                    pt = pt_pool.tile([128, QC], bf16, tag="pt")
                    nc.scalar.activation(pt, st, exp, scale=1.0 / 8.0)
                    res.append(pt)
                return res

            ots = {}

            def emit_pv(pc, pt_, ppts):
                if pt_ == 0:
                    ot_a = ps_ot.tile([HD + 1, QC], fp32, tag="ot")
                    ot_b = ps_ot.tile([HD + 1, QC], fp32, tag="ot")
                    ots[pc] = (ot_a, ot_b)
                for o, pp in zip(ots[pc], ppts):
                    for u in range(QC // MM_N):
                        nc.tensor.matmul(
                            o[:, bass.ts(u, MM_N)], v_sb[:, pt_, :],
                            pp[:, bass.ts(u, MM_N)],
                            start=(pt_ == 0), stop=(pt_ == NKT - 1),
                        )

            otcps = {}

            def emit_otcp(pc):
                # Free the PSUM accumulators fast (one DVE copy each) so the
                # next chunk's PV matmuls aren't blocked behind the slow
                # normalization chain; the divide happens lazily off ot_cp.
                cp_a = otcp_pool.tile([HD + 1, QC], fp32, tag="otcp")
                cp_b = otcp_pool.tile([HD + 1, QC], fp32, tag="otcp")
                nc.vector.tensor_copy(cp_a, ots[pc][0])
                nc.scalar.activation(  # ACT is idle here; halves the wait
                    cp_b, ots[pc][1], mybir.ActivationFunctionType.Copy
                )
                otcps[pc] = (cp_a, cp_b)
                del ots[pc]

            def emit_norm_piece(pc, h, u, use_act=False):
                # ot rows 0..63 / row 64 (denominators): reciprocal, round to
                # bf16, broadcast across the 64 hd partitions with a rank-1
                # PE matmul (ones column), multiply on DVE. The DVE IEEE
                # reciprocal costs ~3.3us per 512 elems, so in steady state it
                # runs off the critical path; in the epilogue (use_act=True)
                # the idle Scalar engine computes 1/d = exp(-ln d) instead.
                o = otcps[pc][h]
                usl = bass.ts(u, MM_N)
                rsb = nrm_pool.tile([1, MM_N], bf16, tag="rsb")
                if use_act:
                    rs = nrm_pool.tile([1, MM_N], fp32, tag="rs")
                    nc.scalar.activation(
                        rs, o[HD:HD + 1, usl],
                        mybir.ActivationFunctionType.Ln,
                    )
                    nc.scalar.activation(rsb, rs, exp, scale=-1.0)
                else:
                    rs = nrm_pool.tile([1, MM_N], fp32, tag="rs")
                    nc.vector.reciprocal(rs, o[HD:HD + 1, usl])
                    nc.vector.tensor_copy(rsb, rs)
                bc_ps = ps_st.tile([HD, MM_N], fp32, tag="st")
                nc.tensor.matmul(bc_ps, ones_col, rsb, start=True, stop=True)
                nc.vector.tensor_mul(
                    ot_full[h * HD:(h + 1) * HD,
                            pc * QC + u * MM_N:pc * QC + (u + 1) * MM_N],
                    o[0:HD, usl], bc_ps,
                )

            def outproj_piece(jq):
                for u2 in range(D // MM_N):
                    yp = ps_st.tile([128, MM_N], fp32, tag="st")
                    nc.tensor.matmul(
                        yp, ot_full[:, bass.ts(jq, 128)],
                        ow_sb[:, bass.ts(u2, MM_N)],
                        start=True, stop=True,
                    )
                    ysb = ysb_pool.tile([128, MM_N], fp32, tag="ysb")
                    nc.vector.tensor_copy(ysb, yp)
                    nc.sync.dma_start(
                        out=y[:, :][bass.ts(jq, 128), bass.ts(u2, MM_N)],
                        in_=ysb,
                    )

            # ---- prologue: all KV slices (xt-DMA paced), Q proj chunk 0.
            # Keeping DMA-dependent work out of the attention loop avoids
            # wait-queue pileups that collapse the PE into its mid p-state.
            for j in range(NDC):
                kv_slice(j)
            proj_slice(qt_sb, qw_sb, qb_sb, 0)
            proj_slice(qt_sb, qw_sb, qb_sb, 1)

            # ---- main software-pipelined loop ----
            # norm pieces (h, u) for the previous chunk run at steps 5-17,
            # spaced so each DVE reciprocal (~3.3us) completes before its
            # PE broadcast matmul is enqueued (no head-of-line blocking);
            # outproj pieces follow once their ot_full half is normalized.
            NORM_STEPS = (5, 9, 13, 17)
            OUTPROJ_STEPS = (11, 13, 15, 17, 19, 21, 23, 25)
            pending_outproj = []
            pending_norm = []
            prev = None
            for s_ in range(NQC * NKT):
                c, t = divmod(s_, NKT)
                pts = emit_scores(c, t)
                if prev is not None:
                    (pc, pt_), ppts = prev
                    emit_pv(pc, pt_, ppts)
                    if pt_ == NKT - 1:
                        emit_otcp(pc)
                        pending_norm = [
                            (pc, h, u) for u in range(2) for h in range(2)
                        ]
                        pending_outproj.extend(
                            range(pc * (QC // 128), (pc + 1) * (QC // 128))
                        )
                # ---- PE filler work (keeps the systolic array saturated) ----
                if pending_norm and t in NORM_STEPS:
                    emit_norm_piece(*pending_norm.pop(0))
                if c + 1 < NQC:
                    if t == 6:
                        proj_slice(qt_sb, qw_sb, qb_sb, 2 * (c + 1))
                    elif t == 22:
                        proj_slice(qt_sb, qw_sb, qb_sb, 2 * (c + 1) + 1)
                if pending_outproj and t in OUTPROJ_STEPS:
                    outproj_piece(pending_outproj.pop(0))
                prev = ((c, t), pts)

            # ---- epilogue: last chunk's PV tail, norm via idle ScalarE,
            # outproj pieces interleaved as their ot_full halves finish ----
            (pc, pt_), ppts = prev
            emit_pv(pc, pt_, ppts)
            emit_otcp(pc)
            jq0 = pc * (QC // 128)
            for u in range(2):
                for h in range(2):
                    emit_norm_piece(pc, h, u, use_act=True)
                for jq in range(jq0 + 4 * u, jq0 + 4 * u + 4):
                    outproj_piece(jq)
            for jq in pending_outproj:
                outproj_piece(jq)
    _split_multi_waits(nc)
    return nc


def _split_multi_waits(nc):
    """This toolchain's walrus accepts at most one sync-wait per datapath
    instruction; move extra waits onto same-engine NoOps placed just before."""
    k = 0
    for f in nc.m.functions:
        for blk in f.blocks:
            out = []
            for inst in blk.instructions:
                si = getattr(inst, "sync_info", None)
                ow_ = list(si.on_wait) if (si and si.on_wait) else []
                if len(ow_) > 1:
                    for w in ow_[:-1]:
                        k += 1
                        nop = bass_rust.InstNoOp(
                            name=f"I-wsplit-{k}", ins=[], outs=[]
                        )
                        nop.engine = inst.engine
                        nop.sync_info = mybir.SyncInfo(
                            on_wait=[w], on_update=[]
                        )
                        out.append(nop)
                    inst.sync_info = mybir.SyncInfo(
                        on_wait=[ow_[-1]], on_update=list(si.on_update or [])
                    )
                out.append(inst)
            blk.instructions = out


def _prep_inputs(X, q_w, q_b, k_w, k_b, v_w, v_b, o_w):
    Xt = np.ascontiguousarray(X.reshape(S, D).T).astype(BF16)
    in_maps = []
    for c in range(NCORES):
        kv = c // (NCORES // KVH)
        qs = slice(c * DQ, (c + 1) * DQ)
        ks = slice(kv * HD, (kv + 1) * HD)
        in_maps.append({
            "xt": Xt,
            "qw": np.ascontiguousarray(q_w[:, qs]).astype(BF16),
            "kvw": np.ascontiguousarray(
                np.concatenate([k_w[:, ks], v_w[:, ks]], axis=1)).astype(BF16),
            "ow": np.ascontiguousarray(o_w[qs, :]).astype(BF16),
            "qb": np.ascontiguousarray(q_b[qs]).reshape(DQ, 1).astype(
                np.float32),
            "kvb": np.ascontiguousarray(
                np.concatenate([k_b[ks], v_b[ks]])).reshape(DKV, 1).astype(
                np.float32),
        })
    return in_maps


def kernel(X, q_w, q_b, k_w, k_b, v_w, v_b, o_w, o_b, **run_kwargs):
    global _COMPILED
    if _COMPILED is None:
        _COMPILED = build_bass()
    in_maps = _prep_inputs(X, q_w, q_b, k_w, k_b, v_w, v_b, o_w)
    res = run_bass_kernel_spmd(
        _COMPILED, in_maps, list(range(NCORES)), **run_kwargs
    )
    parts = [r["y"] for r in res.results]
    out = parts[0].astype(np.float32)
    for p in parts[1:]:
        out = out + p
    out = out + o_b.astype(np.float32)[None, :]
    if run_kwargs:
        return out.reshape(B, S, D), res
    return out.reshape(B, S, D)


# revision 23
# speedup vs baseline: 1.6418x; 1.0070x over previous
"""GQA attention kernel for Trainium2, sharded over 8 NeuronCores.

Problem: X (1, 4096, 1024), H=16 q-heads, KVH=4 kv-heads, head_dim=64.
Sharding: 2 q-heads + their shared kv-head per core (tensor parallel over H).
Each core computes q/k/v projections for its heads, fused flash-style
attention, and its 128-row slice of the output projection -> partial
(4096, 1024) f32, summed on host.

v2: software-pipelined single instruction stream tuned to keep the PE
array continuously busy (it only reaches the 2.4 GHz p-state after ~3us
without stalls; the v1 kernel sat at 1.2 GHz through all of attention):
  - scores(t) are emitted before PV(t-1), so exp(t-1) on the Scalar
    engine overlaps score matmuls and PV never waits on it.
  - KV projection (chunk 0), Q projection (chunk c+1) and the output
    projection (chunk c-1) are spread through the attention loop as PE
    filler work instead of serial prologue/epilogue phases.
  - softmax normalization broadcasts the reciprocal denominators with a
    GpSimd partition_broadcast (v1 round-tripped through DRAM, stalling
    the PE ~20us per chunk).

Layouts on device (per core):
  xt   : X^T            (1024 D, 4096 S)  bf16   (host pre-transposed)
  qt   : Q^T            (128 = 2 heads x 64 d, 4096 q) bf16
  kvt  : [K^T; V^T]     (128 = 64 k-d + 64 v-d, 4096 s) bf16
  kt2  : K^T duplicated into both partition halves
  v    : V natural+ones (128 s-tile, 65) x 32 tiles bf16 (col 64 == 1.0)
  St   : scores^T       (128 k, 1024 q) f32 PSUM  = Kt_tile.T @ Qt
  Pt   : exp(St/8)      (128 k, 1024 q) bf16 SBUF (ScalarE, scale folded)
  Ot   : V_aug.T @ Pt   (65, 1024) f32 PSUM; row 64 = softmax denominators
  y    : partial output (4096, 1024) f32 = (Ot/denoms).T @ o_w[rows]
"""

import sys

import numpy as np

try:
    import concourse.bass as bass
except ImportError:  # grading env may not have concourse on sys.path
    for p in ("/opt/trn_rl_repo", "/root/.axon_site/_ro/trn_rl_repo"):
        if p not in sys.path:
            sys.path.append(p)
    import concourse.bass as bass

import bass_rust
import ml_dtypes
from concourse import mybir
from concourse.bass_utils import run_bass_kernel_spmd
from concourse.masks import make_identity
from concourse.tile import TileContext

BF16 = ml_dtypes.bfloat16

B, S, D = 1, 4096, 1024
H, KVH, HD = 16, 4, 64
NCORES = 8
HPC = H // NCORES          # 2 q heads per core
DQ = HPC * HD              # 128 projected q dims per core
DKV = 2 * HD               # 128 = k head + v head dims
QC = 1024                  # attention q-chunk (2 PSUM banks per score tile)
KT = 128                   # k tile (seq positions per score tile)
NKT = S // KT              # 32
NQC = S // QC              # 4
NDC = D // 128             # 8 contraction chunks for projections
MM_N = 512                 # max matmul free dim (one PSUM bank, f32)

_COMPILED = None


def build_bass():
    nc = bass.Bass()
    fp32 = mybir.dt.float32
    bf16 = mybir.dt.bfloat16

    xt = nc.declare_dram_parameter("xt", [D, S], bf16, isOutput=False)
    qw = nc.declare_dram_parameter("qw", [D, DQ], bf16, isOutput=False)
    kvw = nc.declare_dram_parameter("kvw", [D, DKV], bf16, isOutput=False)
    ow = nc.declare_dram_parameter("ow", [DQ, D], bf16, isOutput=False)
    qb = nc.declare_dram_parameter("qb", [DQ, 1], fp32, isOutput=False)
    kvb = nc.declare_dram_parameter("kvb", [DKV, 1], fp32, isOutput=False)
    y = nc.declare_dram_parameter("y", [S, D], fp32, isOutput=True)

    with TileContext(nc) as tc:
        with (
            tc.tile_pool(name="singles", bufs=1) as singles,
            tc.tile_pool(name="pt_pool", bufs=8) as pt_pool,
            tc.tile_pool(name="nrm", bufs=2) as nrm_pool,
            tc.tile_pool(name="otcp", bufs=2) as otcp_pool,
            tc.tile_pool(name="ysb", bufs=3) as ysb_pool,
            tc.tile_pool(name="ps_st", bufs=2, space="PSUM") as ps_st,
            tc.tile_pool(name="ps_ot", bufs=2, space="PSUM") as ps_ot,
        ):
            # ---- constants / weights ----
            ident = singles.tile([128, 128], bf16)
            make_identity(nc, ident)

            # DMA order matters: the prologue needs xt slices 0-1 and the
            # q/kv weights first; the remaining xt slices pace the chunk-0
            # kv fillers; ow is only needed ~100us in, so it goes last.
            # xt moves as 8 DMAs of [128 part, 4 c-chunks, 1024 cols]: 2 KB
            # contiguous DRAM lines (vs 1 KB at 512 cols) for better
            # per-queue throughput; weights split 2 ways to land early.
            xt_sb = singles.tile([128, NDC, S], bf16)
            xt_re = xt[:, :].rearrange("(c p) s -> p c s", p=128)
            kvw_sb = singles.tile([128, NDC, DKV], bf16)
            kvw_re = kvw[:, :].rearrange("(c p) m -> p c m", p=128)
            qw_sb = singles.tile([128, NDC, DQ], bf16)
            qw_re = qw[:, :].rearrange("(c p) m -> p c m", p=128)
            for half in range(2):
                nc.sync.dma_start(
                    out=kvw_sb[:, bass.ts(half, 4), :],
                    in_=kvw_re[:, bass.ts(half, 4), :],
                )
                nc.sync.dma_start(
                    out=qw_sb[:, bass.ts(half, 4), :],
                    in_=qw_re[:, bass.ts(half, 4), :],
                )
            qb_sb = singles.tile([DQ, 1], fp32)
            nc.sync.dma_start(out=qb_sb, in_=qb[:, :])
            kvb_sb = singles.tile([DKV, 1], fp32)
            nc.sync.dma_start(out=kvb_sb, in_=kvb[:, :])
            for j in range(S // QC):
                for ch in range(2):
                    nc.sync.dma_start(
                        out=xt_sb[:, bass.ts(ch, 4), bass.ts(j, QC)],
                        in_=xt_re[:, bass.ts(ch, 4), bass.ts(j, QC)],
                    )
            ow_sb = singles.tile([DQ, D], bf16)
            nc.sync.dma_start(out=ow_sb, in_=ow[:, :])

            qt_sb = singles.tile([DQ, S], bf16)
            kvt_sb = singles.tile([DKV, S], bf16)
            kt2_sb = singles.tile([DKV, S], bf16)
            v_sb = singles.tile([128, NKT, HD + 1], bf16)
            nc.vector.memset(v_sb, 1.0)
            ot_full = singles.tile([DQ, S], bf16)
            ones_col = singles.tile([1, HD], bf16)
            nc.vector.memset(ones_col, 1.0)

            exp = mybir.ActivationFunctionType.Exp

            def proj_slice(dst, w_sb, b_sb, j):
                ps = ps_st.tile([128, MM_N], fp32, tag="st")
                for c2 in range(NDC):
                    nc.tensor.matmul(
                        ps, w_sb[:, c2, :], xt_sb[:, c2, bass.ts(j, MM_N)],
                        start=(c2 == 0), stop=(c2 == NDC - 1),
                    )
                nc.vector.tensor_scalar_add(
                    dst[:, bass.ts(j, MM_N)], ps, b_sb[:, 0:1]
                )

            def kv_slice(j):
                proj_slice(kvt_sb, kvw_sb, kvb_sb, j)
                nc.sync.dma_start(
                    out=kt2_sb[0:HD, bass.ts(j, MM_N)],
                    in_=kvt_sb[0:HD, bass.ts(j, MM_N)],
                )
                nc.sync.dma_start(
                    out=kt2_sb[HD:DKV, bass.ts(j, MM_N)],
                    in_=kvt_sb[0:HD, bass.ts(j, MM_N)],
                )
                for tt in range(4 * j, 4 * j + 4):
                    pvt = ps_st.tile([128, HD], bf16, tag="st")
                    nc.tensor.transpose(
                        pvt, kvt_sb[HD:DKV, bass.ts(tt, KT)],
                        ident[HD:DKV, HD:DKV],
                    )
                    nc.vector.tensor_copy(v_sb[:, tt, 0:HD], pvt)

            def emit_scores(c, t):
                res = []
                for h in range(HPC):
                    st = ps_st.tile([128, QC], fp32, tag="st")
                    for u in range(QC // MM_N):
                        nc.tensor.matmul(
                            st[:, bass.ts(u, MM_N)],
                            kt2_sb[h * HD:(h + 1) * HD, bass.ts(t, KT)],
                            qt_sb[h * HD:(h + 1) * HD,
                                  c * QC + u * MM_N:c * QC + (u + 1) * MM_N],
                            start=True, stop=True,
                        )
                    pt = pt_pool.tile([128, QC], bf16, tag="pt")
                    nc.scalar.activation(pt, st, exp, scale=1.0 / 8.0)
                    res.append(pt)
                return res

            ots = {}

            def emit_pv(pc, pt_, ppts):
                if pt_ == 0:
                    ot_a = ps_ot.tile([HD + 1, QC], fp32, tag="ot")
                    ot_b = ps_ot.tile([HD + 1, QC], fp32, tag="ot")
                    ots[pc] = (ot_a, ot_b)
                for o, pp in zip(ots[pc], ppts):
                    for u in range(QC // MM_N):
                        nc.tensor.matmul(
                            o[:, bass.ts(u, MM_N)], v_sb[:, pt_, :],
                            pp[:, bass.ts(u, MM_N)],
                            start=(pt_ == 0), stop=(pt_ == NKT - 1),
                        )

            otcps = {}

            def emit_otcp(pc):
                # Free the PSUM accumulators fast (one DVE copy each) so the
                # next chunk's PV matmuls aren't blocked behind the slow
                # normalization chain; the divide happens lazily off ot_cp.
                cp_a = otcp_pool.tile([HD + 1, QC], fp32, tag="otcp")
                cp_b = otcp_pool.tile([HD + 1, QC], fp32, tag="otcp")
                nc.vector.tensor_copy(cp_a, ots[pc][0])
                nc.scalar.activation(  # ACT is idle here; halves the wait
                    cp_b, ots[pc][1], mybir.ActivationFunctionType.Copy
                )
                otcps[pc] = (cp_a, cp_b)
                del ots[pc]

            def emit_norm_piece(pc, h, u, use_act=False):
                # ot rows 0..63 / row 64 (denominators): reciprocal, round to
                # bf16, broadcast across the 64 hd partitions with a rank-1
                # PE matmul (ones column), multiply on DVE. The DVE IEEE
                # reciprocal costs ~3.3us per 512 elems, so in steady state it
                # runs off the critical path; in the epilogue (use_act=True)
                # the idle Scalar engine computes 1/d = exp(-ln d) instead.
                o = otcps[pc][h]
                usl = bass.ts(u, MM_N)
                rsb = nrm_pool.tile([1, MM_N], bf16, tag="rsb")
                if use_act:
                    rs = nrm_pool.tile([1, MM_N], fp32, tag="rs")
                    nc.scalar.activation(
                        rs, o[HD:HD + 1, usl],
                        mybir.ActivationFunctionType.Ln,
                    )
                    nc.scalar.activation(rsb, rs, exp, scale=-1.0)
                else:
                    rs = nrm_pool.tile([1, MM_N], fp32, tag="rs")
                    nc.vector.reciprocal(rs, o[HD:HD + 1, usl])
                    nc.vector.tensor_copy(rsb, rs)
                bc_ps = ps_st.tile([HD, MM_N], fp32, tag="st")
                nc.tensor.matmul(bc_ps, ones_col, rsb, start=True, stop=True)
                nc.vector.tensor_mul(
                    ot_full[h * HD:(h + 1) * HD,
                            pc * QC + u * MM_N:pc * QC + (u + 1) * MM_N],
                    o[0:HD, usl], bc_ps,
                )

            def outproj_piece(jq):
                for u2 in range(D // MM_N):
                    yp = ps_st.tile([128, MM_N], fp32, tag="st")
                    nc.tensor.matmul(
                        yp, ot_full[:, bass.ts(jq, 128)],
                        ow_sb[:, bass.ts(u2, MM_N)],
                        start=True, stop=True,
                    )
                    ysb = ysb_pool.tile([128, MM_N], fp32, tag="ysb")
                    nc.vector.tensor_copy(ysb, yp)
                    nc.sync.dma_start(
                        out=y[:, :][bass.ts(jq, 128), bass.ts(u2, MM_N)],
                        in_=ysb,
                    )

            # ---- prologue: all KV slices (xt-DMA paced), Q proj chunk 0.
            # Keeping DMA-dependent work out of the attention loop avoids
            # wait-queue pileups that collapse the PE into its mid p-state.
            for j in range(NDC):
                kv_slice(j)
            proj_slice(qt_sb, qw_sb, qb_sb, 0)
            proj_slice(qt_sb, qw_sb, qb_sb, 1)

            # ---- main software-pipelined loop ----
            # norm pieces (h, u) for the previous chunk run at steps 5-17,
            # spaced so each DVE reciprocal (~3.3us) completes before its
            # PE broadcast matmul is enqueued (no head-of-line blocking);
            # outproj pieces follow once their ot_full half is normalized.
            NORM_STEPS = (5, 9, 13, 17)
            OUTPROJ_STEPS = (11, 13, 15, 17, 19, 21, 23, 25)
            pending_outproj = []
            pending_norm = []
            prev = None
            for s_ in range(NQC * NKT):
                c, t = divmod(s_, NKT)
                pts = emit_scores(c, t)
                if prev is not None:
                    (pc, pt_), ppts = prev
                    emit_pv(pc, pt_, ppts)
                    if pt_ == NKT - 1:
                        emit_otcp(pc)
                        pending_norm = [
                            (pc, h, u) for u in range(2) for h in range(2)
                        ]
                        pending_outproj.extend(
                            range(pc * (QC // 128), (pc + 1) * (QC // 128))
                        )
                # ---- PE filler work (keeps the systolic array saturated) ----
                if pending_norm and t in NORM_STEPS:
                    emit_norm_piece(*pending_norm.pop(0))
                if c + 1 < NQC:
                    if t == 6:
                        proj_slice(qt_sb, qw_sb, qb_sb, 2 * (c + 1))
                    elif t == 22:
                        proj_slice(qt_sb, qw_sb, qb_sb, 2 * (c + 1) + 1)
                if pending_outproj and t in OUTPROJ_STEPS:
                    outproj_piece(pending_outproj.pop(0))
                prev = ((c, t), pts)

            # ---- epilogue: last chunk's PV tail, norm via idle ScalarE,
            # outproj pieces interleaved as their ot_full halves finish ----
            (pc, pt_), ppts = prev
            emit_pv(pc, pt_, ppts)
            emit_otcp(pc)
            jq0 = pc * (QC // 128)
            for u in range(2):
                # h1 reciprocal on DVE and h0 via ScalarE ln/exp in parallel
                emit_norm_piece(pc, 1, u, use_act=False)
                emit_norm_piece(pc, 0, u, use_act=True)
                for jq in range(jq0 + 4 * u, jq0 + 4 * u + 4):
                    outproj_piece(jq)
            for jq in pending_outproj:
                outproj_piece(jq)
    _split_multi_waits(nc)
    return nc


def _split_multi_waits(nc):
    """This toolchain's walrus accepts at most one sync-wait per datapath
    instruction; move extra waits onto same-engine NoOps placed just before."""
    k = 0
    for f in nc.m.functions:
        for blk in f.blocks:
            out = []
            for inst in blk.instructions:
                si = getattr(inst, "sync_info", None)
                ow_ = list(si.on_wait) if (si and si.on_wait) else []
                if len(ow_) > 1:
                    for w in ow_[:-1]:
                        k += 1
                        nop = bass_rust.InstNoOp(
                            name=f"I-wsplit-{k}", ins=[], outs=[]
                        )
                        nop.engine = inst.engine
                        nop.sync_info = mybir.SyncInfo(
                            on_wait=[w], on_update=[]
                        )
                        out.append(nop)
                    inst.sync_info = mybir.SyncInfo(
                        on_wait=[ow_[-1]], on_update=list(si.on_update or [])
                    )
                out.append(inst)
            blk.instructions = out


def _prep_inputs(X, q_w, q_b, k_w, k_b, v_w, v_b, o_w):
    Xt = np.ascontiguousarray(X.reshape(S, D).T).astype(BF16)
    in_maps = []
    for c in range(NCORES):
        kv = c // (NCORES // KVH)
        qs = slice(c * DQ, (c + 1) * DQ)
        ks = slice(kv * HD, (kv + 1) * HD)
        in_maps.append({
            "xt": Xt,
            "qw": np.ascontiguousarray(q_w[:, qs]).astype(BF16),
            "kvw": np.ascontiguousarray(
                np.concatenate([k_w[:, ks], v_w[:, ks]], axis=1)).astype(BF16),
            "ow": np.ascontiguousarray(o_w[qs, :]).astype(BF16),
            "qb": np.ascontiguousarray(q_b[qs]).reshape(DQ, 1).astype(
                np.float32),
            "kvb": np.ascontiguousarray(
                np.concatenate([k_b[ks], v_b[ks]])).reshape(DKV, 1).astype(
                np.float32),
        })
    return in_maps


def kernel(X, q_w, q_b, k_w, k_b, v_w, v_b, o_w, o_b, **run_kwargs):
    global _COMPILED
    if _COMPILED is None:
        _COMPILED = build_bass()
    in_maps = _prep_inputs(X, q_w, q_b, k_w, k_b, v_w, v_b, o_w)
    res = run_bass_kernel_spmd(
        _COMPILED, in_maps, list(range(NCORES)), **run_kwargs
    )
    parts = [r["y"] for r in res.results]
    out = parts[0].astype(np.float32)
    for p in parts[1:]:
        out = out + p
    out = out + o_b.astype(np.float32)[None, :]
    if run_kwargs:
        return out.reshape(B, S, D), res
    return out.reshape(B, S, D)
